# revision 4
# baseline (speedup 1.0000x reference)
"""Trainium2 Bass kernel v2 for nn_DownModel (GNN message passing).

This execution stack charges ~50us per dynamic instruction (software
sequencer dispatch dominates), so the design minimizes dynamic
instruction count: batched elementwise ops, batched PSUM-macro
transposes, one-hot selector matmuls for the edge segment-sum, and
3 batched edge DMAs.  Row-block sharding over 8 cores with 2
AllGathers (feature|y table, transposed embedding).
"""

import numpy as np

import concourse.bass as bass
import concourse.mybir as mybir
import concourse.tile as tile
from concourse.bass_utils import run_bass_kernel_spmd

F32 = mybir.dt.float32
I32 = mybir.dt.int32
U32 = mybir.dt.uint32

N = 8192
H = 256
C_OUT = 40
K_TOP = 16
NCORES = 8
P = 128
ROWS_PER_CORE = N // NCORES          # 1024
CHUNKS = ROWS_PER_CORE // P          # 8
KT = (2 * H) // P                    # 4 k-tiles of the 2H embedding dim
HKT = H // P                         # 2 k-tiles of the H dim
TBLW = H + C_OUT                     # 296 table row width
SIMW = 512                           # one PSUM bank of fp32
NCC = N // SIMW                      # 16 column chunks
GROUP = 2                            # row-tiles sharing one rhs stream pass
EW = 32                              # gather/scatter offsets per partition


def _split_waits(nc, maxw=1):
    """Walrus accepts one sync-wait per instruction; hoist extras onto NOPs."""
    n_new = 0
    for bb in nc.main_func.blocks:
        new_insts = []
        for ins in bb.instructions:
            si = ins.sync_info
            if si is not None and si.on_wait and len(si.on_wait) > maxw:
                waits = list(si.on_wait)
                excess, keep = waits[:-maxw], waits[-maxw:]
                for i in range(0, len(excess), maxw):
                    nop = mybir.InstNoOp(
                        name=f"waitnop-{ins.name}-{i}",
                        engine=ins.engine,
                        ins=[],
                        outs=[],
                        sync_info=mybir.SyncInfo(
                            on_wait=excess[i:i + maxw], on_update=[]
                        ),
                    )
                    new_insts.append(nop)
                    n_new += 1
                si.on_wait = keep
            new_insts.append(ins)
        bb.instructions[:] = new_insts
    return n_new


def build(EB, repeat=1, phase_lim=6):
    """EB: number of edge gather/scatter batches (each 128*EW edges)."""
    nc = bass.Bass(num_devices=NCORES)

    feat_d = nc.dram_tensor("feat", [ROWS_PER_CORE, H], F32, kind="ExternalInput")
    ecol_d = nc.dram_tensor("ecol", [P, CHUNKS * EB], I32, kind="ExternalInput")
    erow_d = nc.dram_tensor("erow", [P, CHUNKS * EB], F32, kind="ExternalInput")
    eval_d = nc.dram_tensor("eval", [P, CHUNKS * EB], F32, kind="ExternalInput")
    iota_d = nc.dram_tensor("iota", [P, P], F32, kind="ExternalInput")
    cA_d = nc.dram_tensor("cA", [P, H], F32, kind="ExternalInput")
    cB_d = nc.dram_tensor("cB", [P, H], F32, kind="ExternalInput")
    cC_d = nc.dram_tensor("cC", [P, H], F32, kind="ExternalInput")
    bal_d = nc.dram_tensor("bal", [P, 2 * H], F32, kind="ExternalInput")
    bias_d = nc.dram_tensor("bias", [P, C_OUT], F32, kind="ExternalInput")
    gcnw_d = nc.dram_tensor("gcnw", [H, C_OUT], F32, kind="ExternalInput")
    ident_d = nc.dram_tensor("ident", [P, P], F32, kind="ExternalInput")

    out_d = nc.dram_tensor("out", [ROWS_PER_CORE, C_OUT], F32,
                           kind="ExternalOutput")

    with tile.TileContext(nc) as tc:
        with tc.tile_pool(name="consts", bufs=1) as cp, \
             tc.tile_pool(name="persist", bufs=1) as pp, \
             tc.tile_pool(name="dram", bufs=1, space="DRAM") as dp:

            cA = cp.tile([P, H], F32)
            cB = cp.tile([P, H], F32)
            cC = cp.tile([P, H], F32)
            bal = cp.tile([P, 2 * H], F32)
            bias = cp.tile([P, C_OUT], F32)
            gcnw = cp.tile([P, HKT * C_OUT], F32)
            ident = cp.tile([P, P], F32)
            iota = cp.tile([P, P], F32)
            nc.sync.dma_start(out=cA[:], in_=cA_d[:])
            nc.sync.dma_start(out=cB[:], in_=cB_d[:])
            nc.sync.dma_start(out=cC[:], in_=cC_d[:])
            nc.sync.dma_start(out=bal[:], in_=bal_d[:])
            nc.sync.dma_start(out=bias[:], in_=bias_d[:])
            nc.sync.dma_start(
                out=gcnw[:].rearrange("p (k w) -> p k w", k=HKT),
                in_=gcnw_d[:].rearrange("(k p) w -> p k w", p=P))
            nc.sync.dma_start(out=ident[:], in_=ident_d[:])
            nc.sync.dma_start(out=iota[:], in_=iota_d[:])

            agg_all = pp.tile([P, CHUNKS * TBLW], F32)
            embTloc = pp.tile([P, KT * ROWS_PER_CORE], F32)

            for rep in range(repeat):
                table_loc = dp.tile([ROWS_PER_CORE, TBLW], F32,
                                    name=f"table_loc_{rep}")
                table_g = dp.tile([N, TBLW], F32, addr_space="Shared",
                                  name=f"table_g_{rep}")
                embT_loc_d = dp.tile([2 * H, ROWS_PER_CORE], F32,
                                     name=f"embT_loc_d_{rep}")
                embT_g = dp.tile([NCORES * 2 * H, ROWS_PER_CORE], F32,
                                 addr_space="Shared", name=f"embT_g_{rep}")

                _f1ctx = tc.tile_pool(name=f"f1p_{rep}", bufs=1)
                f1p = _f1ctx.__enter__()
                f1_all = f1p.tile([P, CHUNKS * H], F32, name=f"f1_all_{rep}")

                # ===== P1: f1 + y for the local row block =====
                with tc.tile_pool(name=f"p1_{rep}", bufs=1) as p1, \
                     tc.tile_pool(name=f"p1ps_{rep}", bufs=1, space="PSUM") as p1ps:
                    W1 = CHUNKS * H
                    ft = p1.tile([P, W1], F32)
                    nc.sync.dma_start(
                        out=ft[:].rearrange("p (r w) -> p r w", r=CHUNKS),
                        in_=feat_d[:].rearrange("(r p) w -> p r w", p=P))
                    mB = lambda c: c[:, None, :].to_broadcast([P, CHUNKS, H])
                    v3 = lambda t: t[:].rearrange("p (r w) -> p r w", r=CHUNKS)
                    m = p1.tile([P, W1], F32)
                    nc.vector.tensor_tensor(out=v3(m), in0=v3(ft), in1=mB(cB),
                                            op=mybir.AluOpType.mult)
                    nc.vector.tensor_scalar(out=m[:], in0=m[:], scalar1=0.0,
                                            scalar2=None, op0=mybir.AluOpType.max)
                    nc.vector.tensor_tensor(out=v3(m), in0=v3(m), in1=mB(cA),
                                            op=mybir.AluOpType.mult)
                    v = p1.tile([P, W1], F32)
                    nc.vector.tensor_tensor(out=v3(v), in0=v3(ft), in1=mB(cC),
                                            op=mybir.AluOpType.mult)
                    nc.vector.tensor_tensor(out=m[:], in0=m[:], in1=v[:],
                                            op=mybir.AluOpType.add)
                    # elu(z) = (relu(z) - 1) + exp(min(z, 0))
                    nc.vector.tensor_scalar(out=v[:], in0=m[:], scalar1=0.0,
                                            scalar2=-1.0, op0=mybir.AluOpType.max,
                                            op1=mybir.AluOpType.add)
                    nc.vector.tensor_scalar(out=m[:], in0=m[:], scalar1=0.0,
                                            scalar2=None, op0=mybir.AluOpType.min)
                    nc.scalar.activation(out=m[:], in_=m[:],
                                         func=mybir.ActivationFunctionType.Exp)
                    nc.vector.tensor_tensor(out=f1_all[:], in0=v[:], in1=m[:],
                                            op=mybir.AluOpType.add)
                    nc.sync.dma_start(
                        out=table_loc[:, 0:H].rearrange("(r p) w -> p r w", p=P),
                        in_=v3(f1_all))
                    # y = f1 @ gcn_w : 16 transposes into one PSUM macro tile
                    psT = p1ps.tile([P, 2048], F32, space="PSUM")
                    for rt in range(CHUNKS):
                        for k in range(HKT):
                            nc.tensor.transpose(
                                out=psT[:, (rt * HKT + k) * P:
                                        (rt * HKT + k + 1) * P],
                                in_=f1_all[:, rt * H + k * P:rt * H + (k + 1) * P],
                                identity=ident[:])
                    f1T = p1.tile([P, CHUNKS * H], F32)
                    nc.scalar.copy(out=f1T[:], in_=psT[:])
                    psy = p1ps.tile([P, CHUNKS * C_OUT], F32, space="PSUM")
                    for rt in range(CHUNKS):
                        for k in range(HKT):
                            nc.tensor.matmul(
                                out=psy[:, rt * C_OUT:(rt + 1) * C_OUT],
                                lhsT=f1T[:, (rt * HKT + k) * P:
                                         (rt * HKT + k + 1) * P],
                                rhs=gcnw[:, k * C_OUT:(k + 1) * C_OUT],
                                start=(k == 0), stop=(k == HKT - 1))
                    yt = p1.tile([P, CHUNKS * C_OUT], F32)
                    nc.scalar.copy(out=yt[:], in_=psy[:])
                    nc.sync.dma_start(
                        out=table_loc[:, H:TBLW].rearrange("(r p) w -> p r w", p=P),
                        in_=yt[:].rearrange("p (r w) -> p r w", r=CHUNKS))

                if phase_lim >= 2:
                    # ===== AG1: all-gather the [N, H+C] table =====
                    nc.gpsimd.collective_compute(
                        "AllGather", mybir.AluOpType.bypass,
                        replica_groups=[list(range(NCORES))],
                        ins=[table_loc.opt()], outs=[table_g.opt()],
                    )

                if phase_lim >= 3:
                    # ===== P2: edge segment-sum via one-hot selector matmuls =====
                    with tc.tile_pool(name=f"p2e_{rep}", bufs=1) as p2e, \
                         tc.tile_pool(name=f"p2g_{rep}", bufs=2) as p2g, \
                         tc.tile_pool(name=f"p2s_{rep}", bufs=2) as p2s, \
                         tc.tile_pool(name=f"p2ps_{rep}", bufs=2, space="PSUM") as p2ps:
                        T = EB
                        er = p2e.tile([P, CHUNKS * T], F32)
                        ec = p2e.tile([P, CHUNKS * T], I32)
                        ev = p2e.tile([P, CHUNKS * T], F32)
                        nc.sync.dma_start(out=er[:], in_=erow_d[:])
                        nc.sync.dma_start(out=ec[:], in_=ecol_d[:])
                        nc.sync.dma_start(out=ev[:], in_=eval_d[:])
                        for ci in range(CHUNKS):
                            # one-hot selector blocks for all T tiles: 2 ops
                            S_all = p2s.tile([P, T * P], F32, tag="sall",
                                             name=f"S_{ci}")
                            S3 = S_all[:].rearrange("p (t r) -> p t r", t=T)
                            nc.vector.tensor_tensor(
                                out=S3,
                                in0=er[:, ci * T:(ci + 1) * T, None]
                                    .to_broadcast([P, T, P]),
                                in1=iota[:, None, :].to_broadcast([P, T, P]),
                                op=mybir.AluOpType.is_equal)
                            nc.vector.tensor_tensor(
                                out=S3, in0=S3,
                                in1=ev[:, ci * T:(ci + 1) * T, None]
                                    .to_broadcast([P, T, P]),
                                op=mybir.AluOpType.mult)
                            psa = p2ps.tile([P, TBLW], F32, space="PSUM")
                            gb = p2g.tile([P, T * TBLW], F32, tag="gtile",
                                          name=f"g_{ci}")
                            for t in range(T):
                                nc.gpsimd.indirect_dma_start(
                                    out=gb[:, t * TBLW:(t + 1) * TBLW],
                                    out_offset=None,
                                    in_=table_g[:, :],
                                    in_offset=bass.IndirectOffsetOnAxis(
                                        ap=ec[:, ci * T + t:ci * T + t + 1],
                                        axis=0),
                                )
                            for t in range(T):
                                nc.tensor.matmul(out=psa[:],
                                                 lhsT=S_all[:, t * P:(t + 1) * P],
                                                 rhs=gb[:, t * TBLW:(t + 1) * TBLW],
                                                 start=(t == 0), stop=(t == T - 1))
                            nc.scalar.copy(
                                out=agg_all[:, ci * TBLW:(ci + 1) * TBLW],
                                in_=psa[:])

                if phase_lim >= 4:
                    # ===== P3: embedding build + transpose =====
                    with tc.tile_pool(name=f"p3_{rep}", bufs=1) as p3, \
                         tc.tile_pool(name=f"p3ps_{rep}", bufs=2, space="PSUM") as p3ps:
                        W3 = CHUNKS * 2 * H
                        zc = p3.tile([P, W3], F32)
                        zc3 = zc[:].rearrange("p (r w) -> p r w", r=CHUNKS)
                        nc.vector.tensor_tensor(
                            out=zc3[:, :, 0:H],
                            in0=f1_all[:].rearrange("p (r w) -> p r w", r=CHUNKS),
                            in1=bal[:, None, 0:H].to_broadcast([P, CHUNKS, H]),
                            op=mybir.AluOpType.mult)
                        nc.vector.tensor_tensor(
                            out=zc3[:, :, H:2 * H],
                            in0=agg_all[:].rearrange(
                                "p (r w) -> p r w", r=CHUNKS)[:, :, 0:H],
                            in1=bal[:, None, H:2 * H].to_broadcast([P, CHUNKS, H]),
                            op=mybir.AluOpType.mult)
                        sq = p3.tile([P, 2 * H], F32)
                        n2 = p3.tile([P, CHUNKS], F32)
                        for rt in range(CHUNKS):
                            nc.scalar.activation(
                                out=sq[:],
                                in_=zc[:, rt * 2 * H:(rt + 1) * 2 * H],
                                func=mybir.ActivationFunctionType.Square,
                                accum_out=n2[:, rt:rt + 1])
                        nc.scalar.sqrt(out=n2[:], in_=n2[:])
                        nc.vector.tensor_scalar(out=n2[:], in0=n2[:], scalar1=1e-8,
                                                scalar2=None, op0=mybir.AluOpType.add)
                        inv = p3.tile([P, CHUNKS], F32)
                        nc.vector.reciprocal(out=inv[:], in_=n2[:])
                        nc.vector.tensor_tensor(
                            out=zc3, in0=zc3,
                            in1=inv[:, :, None].to_broadcast([P, CHUNKS, 2 * H]),
                            op=mybir.AluOpType.mult)
                        # 32 transposes -> 2 PSUM macro tiles -> embTloc [p,k,r]
                        for half in range(2):
                            psT = p3ps.tile([P, 2048], F32, space="PSUM")
                            for j in range(4):
                                rt = half * 4 + j
                                for k in range(KT):
                                    nc.tensor.transpose(
                                        out=psT[:, (j * KT + k) * P:
                                                (j * KT + k + 1) * P],
                                        in_=zc[:, rt * 2 * H + k * P:
                                               rt * 2 * H + (k + 1) * P],
                                        identity=ident[:])
                            # psT layout [p, (j, k), 128] -> embTloc [p, k, rt*128]
                            dst = embTloc[:].rearrange(
                                "p (k r) -> p k r", k=KT)[
                                :, :, half * 4 * P:(half + 1) * 4 * P]
                            nc.scalar.copy(
                                out=dst.rearrange("p k (j w) -> p k j w", j=4),
                                in_=psT[:].rearrange(
                                    "p (j k w) -> p k j w", j=4, k=KT))
                        nc.sync.dma_start(
                            out=embT_loc_d[:].rearrange("(k p) r -> p k r", p=P),
                            in_=embTloc[:].rearrange("p (k r) -> p k r", k=KT))

                    # ===== AG2: all-gather transposed embedding =====
                    nc.gpsimd.collective_compute(
                        "AllGather", mybir.AluOpType.bypass,
                        replica_groups=[list(range(NCORES))],
                        ins=[embT_loc_d.opt()], outs=[embT_g.opt()],
                    )

                _f1ctx.__exit__(None, None, None)

                if phase_lim >= 5:
                    # ===== P4+P5: sim row-block, top-k, combine =====
                    ngroups = CHUNKS // GROUP
                    with tc.tile_pool(name=f"p4rhs_{rep}", bufs=1) as p4rhs, \
                         tc.tile_pool(name=f"p4sim_{rep}", bufs=GROUP) as p4sim, \
                         tc.tile_pool(name=f"p4s_{rep}", bufs=1) as p4s, \
                         tc.tile_pool(name=f"p4ps_{rep}", bufs=1, space="PSUM") as p4ps:
                        m16a = p4s.tile([P, CHUNKS * K_TOP], F32)
                        i16a = p4s.tile([P, CHUNKS * K_TOP], U32)
                        for g in range(ngroups):
                            rts = [g * GROUP + j for j in range(GROUP)]
                            sims = {rt: p4sim.tile([P, N], F32, tag="simbuf",
                                                   name=f"sim_rt{rt}")
                                    for rt in rts}
                            for cg in range(2):
                                # rhs: 4096 sim columns = 4 blocks of embT_g
                                rh = p4rhs.tile([P, 4 * KT * ROWS_PER_CORE], F32,
                                                tag="rhs", name=f"rh{g}_{cg}")
                                nc.sync.dma_start(
                                    out=rh[:].rearrange(
                                        "p (b k w) -> p b k w", b=4, k=KT),
                                    in_=embT_g[cg * 8 * H:(cg + 1) * 8 * H, :]
                                        .rearrange("(b k p) w -> p b k w",
                                                   b=4, p=P))
                                rh4 = rh[:].rearrange(
                                    "p (b k w) -> p b k w", b=4, k=KT)
                                for rt in rts:
                                    pss = p4ps.tile([P, 4096], F32,
                                                    space="PSUM", tag="pss",
                                                    name=f"pss{rt}")
                                    for ccq in range(8):
                                        b, off = ccq // 2, (ccq % 2) * SIMW
                                        for k in range(KT):
                                            nc.tensor.matmul(
                                                out=pss[:, ccq * SIMW:
                                                        (ccq + 1) * SIMW],
                                                lhsT=embTloc[
                                                    :, k * ROWS_PER_CORE + rt * P:
                                                    k * ROWS_PER_CORE + (rt + 1) * P],
                                                rhs=rh4[:, b, k, off:off + SIMW],
                                                start=(k == 0), stop=(k == KT - 1))
                                    nc.scalar.copy(
                                        out=sims[rt][:, cg * 4096:(cg + 1) * 4096],
                                        in_=pss[:])
                            for rt in rts:
                                sim = sims[rt]
                                if phase_lim < 6:
                                    nc.vector.max(out=m16a[:, rt * K_TOP:
                                                          rt * K_TOP + 8],
                                                  in_=sim[:])
                                    continue
                                m16 = m16a[:, rt * K_TOP:(rt + 1) * K_TOP]
                                i16 = i16a[:, rt * K_TOP:(rt + 1) * K_TOP]
                                nc.vector.max(out=m16[:, 0:8], in_=sim[:])
                                nc.vector.max_index(out=i16[:, 0:8],
                                                    in_max=m16[:, 0:8],
                                                    in_values=sim[:])
                                nc.vector.match_replace(out=sim[:],
                                                        in_to_replace=m16[:, 0:8],
                                                        in_values=sim[:],
                                                        imm_value=-1e30)
                                nc.vector.max(out=m16[:, 8:16], in_=sim[:])
                                nc.vector.max_index(out=i16[:, 8:16],
                                                    in_max=m16[:, 8:16],
                                                    in_values=sim[:])

                        if phase_lim >= 6:
                            # ===== P5: out = relu(0.5*(Ay + sum relu(v)*y[idx]) + b)
                            v16 = p4s.tile([P, CHUNKS * K_TOP], F32)
                            nc.vector.tensor_scalar(out=v16[:], in0=m16a[:],
                                                    scalar1=0.0, scalar2=None,
                                                    op0=mybir.AluOpType.max)
                            ot = p4s.tile([P, CHUNKS * C_OUT], F32)
                            HG = CHUNKS // 2
                            for hh in range(2):
                                y16 = p4s.tile([P, HG * K_TOP * C_OUT], F32,
                                               tag="y16", name=f"y16_{hh}")
                                for jj in range(HG * K_TOP):
                                    j = hh * HG * K_TOP + jj
                                    nc.gpsimd.indirect_dma_start(
                                        out=y16[:, jj * C_OUT:(jj + 1) * C_OUT],
                                        out_offset=None,
                                        in_=table_g[:, :],
                                        in_offset=bass.IndirectOffsetOnAxis(
                                            ap=i16a[:, j:j + 1], axis=0),
                                        element_offset=H,
                                    )
                                nc.vector.tensor_tensor(
                                    out=y16[:].rearrange("p (a b) -> p a b",
                                                         a=HG * K_TOP),
                                    in0=y16[:].rearrange("p (a b) -> p a b",
                                                         a=HG * K_TOP),
                                    in1=v16[:, hh * HG * K_TOP:
                                            (hh + 1) * HG * K_TOP, None]
                                        .to_broadcast([P, HG * K_TOP, C_OUT]),
                                    op=mybir.AluOpType.mult)
                                for q in range(HG):
                                    rt = hh * HG + q
                                    nc.vector.tensor_reduce(
                                        out=ot[:, rt * C_OUT:(rt + 1) * C_OUT],
                                        in_=y16[:, q * K_TOP * C_OUT:
                                                (q + 1) * K_TOP * C_OUT]
                                            .rearrange("p (a b) -> p b a",
                                                       a=K_TOP),
                                        axis=mybir.AxisListType.X,
                                        op=mybir.AluOpType.add)
                            o3 = ot[:].rearrange("p (r w) -> p r w", r=CHUNKS)
                            nc.vector.tensor_tensor(
                                out=o3, in0=o3,
                                in1=agg_all[:].rearrange(
                                    "p (r w) -> p r w", r=CHUNKS)[:, :, H:TBLW],
                                op=mybir.AluOpType.add)
                            nc.vector.tensor_scalar(out=ot[:], in0=ot[:],
                                                    scalar1=0.5, scalar2=None,
                                                    op0=mybir.AluOpType.mult)
                            nc.vector.tensor_tensor(
                                out=o3, in0=o3,
                                in1=bias[:, None, :].to_broadcast(
                                    [P, CHUNKS, C_OUT]),
                                op=mybir.AluOpType.add)
                            nc.vector.tensor_scalar(out=ot[:], in0=ot[:],
                                                    scalar1=0.0, scalar2=None,
                                                    op0=mybir.AluOpType.max)
                            nc.sync.dma_start(
                                out=out_d[:].rearrange("(r p) w -> p r w", p=P),
                                in_=o3)

            if phase_lim < 6:
                with tc.tile_pool(name="dummyout", bufs=1) as dop:
                    zz = dop.tile([P, C_OUT], F32)
                    nc.vector.memset(zz[:], 0.0)
                    nc.sync.dma_start(
                        out=out_d[:].rearrange("(r p) w -> p r w", p=P),
                        in_=zz[:, None, :].to_broadcast([P, CHUNKS, C_OUT]))

    return nc


def prep_inputs(features, adj_rows, adj_cols, adj_vals, tokens, wp_weight,
                global_token, pre_token_w, combine_w, balance_w, gcn_w, gcn_b):
    """Host-side sharding: row-block features, bucket edges by destination
    row chunk into [P, CHUNKS*T] tiles (edge t*P+p of chunk ci at
    [p, ci*T+t]), pre-broadcast small parameters."""
    features = np.ascontiguousarray(np.asarray(features, dtype=np.float32))
    r = np.asarray(adj_rows).astype(np.int64)
    c = np.asarray(adj_cols).astype(np.int64)
    v = np.asarray(adj_vals, dtype=np.float32)

    pre_token = (np.asarray(wp_weight, np.float32) @
                 np.asarray(tokens, np.float32)).reshape(-1)
    cw = np.asarray(combine_w, np.float32).reshape(-1)
    cA = (cw[0] * np.asarray(global_token, np.float32)).reshape(-1)
    cB = pre_token
    cC = (cw[1] * np.asarray(pre_token_w, np.float32)).reshape(-1)
    bal = np.asarray(balance_w, np.float32).reshape(-1)
    bias = np.asarray(gcn_b, np.float32).reshape(-1)

    bcast = lambda x: np.ascontiguousarray(np.tile(x[None, :], (P, 1)))
    gcnw = np.ascontiguousarray(np.asarray(gcn_w, np.float32))
    iota = np.tile(np.arange(P, dtype=np.float32)[None, :], (P, 1))
    ident = np.eye(P, dtype=np.float32)

    gchunk = r // P
    order = np.argsort(gchunk, kind="stable")
    rs, cs, vs = r[order], c[order], v[order]
    gs = gchunk[order]
    counts = np.bincount(gs, minlength=N // P)
    T = max(1, int(np.ceil(counts.max() / P)))

    erow = np.full((NCORES, CHUNKS, T * P), -1.0, dtype=np.float32)
    ecol = np.zeros((NCORES, CHUNKS, T * P), dtype=np.int32)
    evalv = np.zeros((NCORES, CHUNKS, T * P), dtype=np.float32)
    starts = np.concatenate([[0], np.cumsum(counts)])
    for g in range(N // P):
        core, ci = g // CHUNKS, g % CHUNKS
        s, e = starts[g], starts[g + 1]
        cnt = e - s
        erow[core, ci, :cnt] = (rs[s:e] % P).astype(np.float32)
        ecol[core, ci, :cnt] = cs[s:e].astype(np.int32)
        evalv[core, ci, :cnt] = vs[s:e]

    # [NCORES, CHUNKS, T*P] -> [NCORES, P, CHUNKS*T]: edge t*P+p -> [p, ci*T+t]
    def shuffle(a):
        return np.ascontiguousarray(
            a.reshape(NCORES, CHUNKS, T, P).transpose(0, 3, 1, 2)
             .reshape(NCORES, P, CHUNKS * T))
    erow, ecol, evalv = shuffle(erow), shuffle(ecol), shuffle(evalv)

    in_maps = []
    for core in range(NCORES):
        in_maps.append({
            "feat": features[core * ROWS_PER_CORE:(core + 1) * ROWS_PER_CORE],
            "ecol": ecol[core], "erow": erow[core], "eval": evalv[core],
            "cA": bcast(cA), "cB": bcast(cB), "cC": bcast(cC),
            "bal": bcast(bal), "bias": bcast(bias),
            "gcnw": gcnw, "iota": iota, "ident": ident,
        })
    return in_maps, T


_BUILD_CACHE = {}


def kernel(features, adj_rows, adj_cols, adj_vals, down_k,
           tokens, wp_weight, global_token, pre_token_w, combine_w,
           balance_w, gcn_w, gcn_b):
    k = int(np.asarray(down_k))
    assert k == K_TOP, f"kernel hardcodes top-k={K_TOP}, got {k}"
    in_maps, T = prep_inputs(features, adj_rows, adj_cols, adj_vals, tokens,
                             wp_weight, global_token, pre_token_w, combine_w,
                             balance_w, gcn_w, gcn_b)
    if T not in _BUILD_CACHE:
        nc_new = build(T)
        _split_waits(nc_new)
        _BUILD_CACHE[T] = nc_new
    nc = _BUILD_CACHE[T]
    res = run_bass_kernel_spmd(nc, in_maps, list(range(NCORES)))
    out = np.concatenate([res.results[i]["out"] for i in range(NCORES)], axis=0)
    return out.astype(np.float32)


# revision 5
# speedup vs baseline: 1.5055x; 1.5055x over previous
"""Trainium2 Bass kernel v4 for nn_DownModel (GNN message passing).

This stack charges ~50us per dynamic instruction (software sequencer),
so the design minimizes dynamic instruction count.  v4: the edge
segment-sum is a dense transposed SpMM  aggT = T^T @ A^T  (A^T shipped
dense from the host, 384 matmuls, no per-edge gathers), which also
yields the aggregate already transposed so the embedding/normalization
runs entirely in transposed layout (norms via ones-vector matmuls).
Row-block sharding over 8 cores; 2 AllGathers (table, embT).
"""

import numpy as np

import concourse.bass as bass
import concourse.mybir as mybir
import concourse.tile as tile
from concourse.bass_utils import run_bass_kernel_spmd

F32 = mybir.dt.float32
I32 = mybir.dt.int32
U32 = mybir.dt.uint32

N = 8192
H = 256
C_OUT = 40
K_TOP = 16
NCORES = 8
P = 128
ROWS_PER_CORE = N // NCORES          # 1024
CHUNKS = ROWS_PER_CORE // P          # 8
KT = (2 * H) // P                    # 4 k-tiles of the 2H embedding dim
HKT = H // P                         # 2 k-tiles of the H dim
TBLW = H + C_OUT                     # 296 table row width
SIMW = 512                           # one PSUM bank of fp32
GROUP = 2                            # row-tiles sharing one rhs stream pass
NKT = N // P                         # 64 k-tiles over all nodes
ABAT = 8                             # A^T k-tiles streamed per DMA


def _split_waits(nc, maxw=1):
    """Walrus accepts one sync-wait per instruction; hoist extras onto NOPs."""
    n_new = 0
    for bb in nc.main_func.blocks:
        new_insts = []
        for ins in bb.instructions:
            si = ins.sync_info
            if si is not None and si.on_wait and len(si.on_wait) > maxw:
                waits = list(si.on_wait)
                excess, keep = waits[:-maxw], waits[-maxw:]
                for i in range(0, len(excess), maxw):
                    nop = mybir.InstNoOp(
                        name=f"waitnop-{ins.name}-{i}",
                        engine=ins.engine,
                        ins=[],
                        outs=[],
                        sync_info=mybir.SyncInfo(
                            on_wait=excess[i:i + maxw], on_update=[]
                        ),
                    )
                    new_insts.append(nop)
                    n_new += 1
                si.on_wait = keep
            new_insts.append(ins)
        bb.instructions[:] = new_insts
    return n_new


def build(TC, repeat=1, phase_lim=6):
    """TC: edge tiles per column k-tile for the one-time dense A^T build."""
    nc = bass.Bass(num_devices=NCORES)

    feat_d = nc.dram_tensor("feat", [ROWS_PER_CORE, H], F32, kind="ExternalInput")
    bec_d = nc.dram_tensor("bec", [P, NKT * TC], F32, kind="ExternalInput")
    ber_d = nc.dram_tensor("ber", [P, NKT * TC], F32, kind="ExternalInput")
    bev_d = nc.dram_tensor("bev", [P, NKT * TC], F32, kind="ExternalInput")
    iota_d = nc.dram_tensor("iota", [P, P], F32, kind="ExternalInput")
    iota1k_d = nc.dram_tensor("iota1k", [P, ROWS_PER_CORE], F32,
                              kind="ExternalInput")
    cA_d = nc.dram_tensor("cA", [P, H], F32, kind="ExternalInput")
    cB_d = nc.dram_tensor("cB", [P, H], F32, kind="ExternalInput")
    cC_d = nc.dram_tensor("cC", [P, H], F32, kind="ExternalInput")
    balT_d = nc.dram_tensor("balT", [P, KT], F32, kind="ExternalInput")
    bias_d = nc.dram_tensor("bias", [P, C_OUT], F32, kind="ExternalInput")
    gcnw_d = nc.dram_tensor("gcnw", [H, C_OUT], F32, kind="ExternalInput")
    ident_d = nc.dram_tensor("ident", [P, P], F32, kind="ExternalInput")
    ones_d = nc.dram_tensor("ones", [P, P], F32, kind="ExternalInput")

    out_d = nc.dram_tensor("out", [ROWS_PER_CORE, C_OUT], F32,
                           kind="ExternalOutput")

    with tile.TileContext(nc) as tc:
        with tc.tile_pool(name="consts", bufs=1) as cp, \
             tc.tile_pool(name="persist", bufs=1) as pp, \
             tc.tile_pool(name="dram", bufs=1, space="DRAM") as dp:

            cA = cp.tile([P, H], F32)
            cB = cp.tile([P, H], F32)
            cC = cp.tile([P, H], F32)
            balT = cp.tile([P, KT], F32)
            bias = cp.tile([P, C_OUT], F32)
            gcnw = cp.tile([P, HKT * C_OUT], F32)
            ident = cp.tile([P, P], F32)
            ones = cp.tile([P, P], F32)
            nc.sync.dma_start(out=cA[:], in_=cA_d[:])
            nc.sync.dma_start(out=cB[:], in_=cB_d[:])
            nc.sync.dma_start(out=cC[:], in_=cC_d[:])
            nc.sync.dma_start(out=balT[:], in_=balT_d[:])
            nc.sync.dma_start(out=bias[:], in_=bias_d[:])
            nc.sync.dma_start(
                out=gcnw[:].rearrange("p (k w) -> p k w", k=HKT),
                in_=gcnw_d[:].rearrange("(k p) w -> p k w", p=P))
            nc.sync.dma_start(out=ident[:], in_=ident_d[:])
            nc.sync.dma_start(out=ones[:], in_=ones_d[:])

            embTloc = pp.tile([P, KT * ROWS_PER_CORE], F32)
            ot1 = pp.tile([P, CHUNKS * C_OUT], F32)   # A@y, row-major

            # ===== one-time: dense A^T built on device from one-hot edges
            # (outside the repeat loop; constant across reps) =====
            at_dev = dp.tile([N, ROWS_PER_CORE], F32, name="at_dev")
            with tc.tile_pool(name="bld", bufs=1) as bp, \
                 tc.tile_pool(name="bldps", bufs=2, space="PSUM") as bps:
                bec = bp.tile([P, NKT * TC], F32)
                ber = bp.tile([P, NKT * TC], F32)
                bev = bp.tile([P, NKT * TC], F32)
                iota = bp.tile([P, P], F32)
                iota1k = bp.tile([P, ROWS_PER_CORE], F32)
                nc.sync.dma_start(out=bec[:], in_=bec_d[:])
                nc.sync.dma_start(out=ber[:], in_=ber_d[:])
                nc.sync.dma_start(out=bev[:], in_=bev_d[:])
                nc.sync.dma_start(out=iota[:], in_=iota_d[:])
                nc.sync.dma_start(out=iota1k[:], in_=iota1k_d[:])
                for ct in range(NKT):
                    Sc = bp.tile([P, TC * P], F32, tag="sc", name=f"sc{ct}")
                    nc.vector.tensor_tensor(
                        out=Sc[:].rearrange("p (t c) -> p t c", t=TC),
                        in0=bec[:, ct * TC:(ct + 1) * TC, None]
                            .to_broadcast([P, TC, P]),
                        in1=iota[:, None, :].to_broadcast([P, TC, P]),
                        op=mybir.AluOpType.is_equal)
                    Rv = bp.tile([P, TC * ROWS_PER_CORE], F32, tag="rv",
                                 name=f"rv{ct}")
                    R3 = Rv[:].rearrange("p (t r) -> p t r", t=TC)
                    nc.vector.tensor_tensor(
                        out=R3,
                        in0=ber[:, ct * TC:(ct + 1) * TC, None]
                            .to_broadcast([P, TC, ROWS_PER_CORE]),
                        in1=iota1k[:, None, :].to_broadcast(
                            [P, TC, ROWS_PER_CORE]),
                        op=mybir.AluOpType.is_equal)
                    nc.vector.tensor_tensor(
                        out=R3, in0=R3,
                        in1=bev[:, ct * TC:(ct + 1) * TC, None]
                            .to_broadcast([P, TC, ROWS_PER_CORE]),
                        op=mybir.AluOpType.mult)
                    psA = bps.tile([P, ROWS_PER_CORE], F32, space="PSUM",
                                   tag="psA", name=f"psA{ct}")
                    for t in range(TC):
                        for hf in range(2):
                            nc.tensor.matmul(
                                out=psA[:, hf * SIMW:(hf + 1) * SIMW],
                                lhsT=Sc[:, t * P:(t + 1) * P],
                                rhs=Rv[:, t * ROWS_PER_CORE + hf * SIMW:
                                       t * ROWS_PER_CORE + hf * SIMW + SIMW],
                                start=(t == 0), stop=(t == TC - 1))
                    ab = bp.tile([P, ROWS_PER_CORE], F32, tag="ab",
                                 name=f"ab{ct}")
                    nc.scalar.copy(out=ab[:], in_=psA[:])
                    nc.sync.dma_start(out=at_dev[ct * P:(ct + 1) * P, :],
                                      in_=ab[:])

            for rep in range(repeat):
                table_loc = dp.tile([ROWS_PER_CORE, TBLW], F32,
                                    name=f"table_loc_{rep}")
                table_g = dp.tile([N, TBLW], F32, addr_space="Shared",
                                  name=f"table_g_{rep}")
                embT_loc_d = dp.tile([2 * H, ROWS_PER_CORE], F32,
                                     name=f"embT_loc_d_{rep}")
                embT_g = dp.tile([NCORES * 2 * H, ROWS_PER_CORE], F32,
                                 addr_space="Shared", name=f"embT_g_{rep}")

                _sctx = tc.tile_pool(name=f"sp_{rep}", bufs=1)
                sp = _sctx.__enter__()
                f1T = sp.tile([P, HKT * ROWS_PER_CORE], F32,
                              name=f"f1T_{rep}")     # [p, k(2), r(1024)]

                # ===== P1: f1 (row-major) + f1T + y for the local block =====
                with tc.tile_pool(name=f"p1_{rep}", bufs=1) as p1, \
                     tc.tile_pool(name=f"p1ps_{rep}", bufs=1, space="PSUM") as p1ps:
                    W1 = CHUNKS * H
                    ft = p1.tile([P, W1], F32)
                    nc.sync.dma_start(
                        out=ft[:].rearrange("p (r w) -> p r w", r=CHUNKS),
                        in_=feat_d[:].rearrange("(r p) w -> p r w", p=P))
                    mB = lambda c: c[:, None, :].to_broadcast([P, CHUNKS, H])
                    v3 = lambda t: t[:].rearrange("p (r w) -> p r w", r=CHUNKS)
                    m = p1.tile([P, W1], F32)
                    nc.vector.tensor_tensor(out=v3(m), in0=v3(ft), in1=mB(cB),
                                            op=mybir.AluOpType.mult)
                    nc.vector.tensor_scalar(out=m[:], in0=m[:], scalar1=0.0,
                                            scalar2=None, op0=mybir.AluOpType.max)
                    nc.vector.tensor_tensor(out=v3(m), in0=v3(m), in1=mB(cA),
                                            op=mybir.AluOpType.mult)
                    v = p1.tile([P, W1], F32)
                    nc.vector.tensor_tensor(out=v3(v), in0=v3(ft), in1=mB(cC),
                                            op=mybir.AluOpType.mult)
                    nc.vector.tensor_tensor(out=m[:], in0=m[:], in1=v[:],
                                            op=mybir.AluOpType.add)
                    # elu(z) = (relu(z) - 1) + exp(min(z, 0))
                    nc.vector.tensor_scalar(out=v[:], in0=m[:], scalar1=0.0,
                                            scalar2=-1.0, op0=mybir.AluOpType.max,
                                            op1=mybir.AluOpType.add)
                    nc.vector.tensor_scalar(out=m[:], in0=m[:], scalar1=0.0,
                                            scalar2=None, op0=mybir.AluOpType.min)
                    nc.scalar.activation(out=m[:], in_=m[:],
                                         func=mybir.ActivationFunctionType.Exp)
                    f1_all = p1.tile([P, W1], F32)
                    nc.vector.tensor_tensor(out=f1_all[:], in0=v[:], in1=m[:],
                                            op=mybir.AluOpType.add)
                    nc.sync.dma_start(
                        out=table_loc[:, 0:H].rearrange("(r p) w -> p r w", p=P),
                        in_=v3(f1_all))
                    # 16 transposes -> f1T in [p, k, rt*128+c] layout
                    psT = p1ps.tile([P, 2048], F32, space="PSUM")
                    for rt in range(CHUNKS):
                        for k in range(HKT):
                            nc.tensor.transpose(
                                out=psT[:, (rt * HKT + k) * P:
                                        (rt * HKT + k + 1) * P],
                                in_=f1_all[:, rt * H + k * P:rt * H + (k + 1) * P],
                                identity=ident[:])
                    nc.scalar.copy(
                        out=f1T[:].rearrange("p (k r c) -> p r k c",
                                             k=HKT, r=CHUNKS),
                        in_=psT[:].rearrange("p (r k c) -> p r k c",
                                             r=CHUNKS, k=HKT))
                    # y = f1 @ gcn_w
                    psy = p1ps.tile([P, CHUNKS * C_OUT], F32, space="PSUM")
                    for rt in range(CHUNKS):
                        for k in range(HKT):
                            nc.tensor.matmul(
                                out=psy[:, rt * C_OUT:(rt + 1) * C_OUT],
                                lhsT=f1T[:, k * ROWS_PER_CORE + rt * P:
                                         k * ROWS_PER_CORE + (rt + 1) * P],
                                rhs=gcnw[:, k * C_OUT:(k + 1) * C_OUT],
                                start=(k == 0), stop=(k == HKT - 1))
                    yt = p1.tile([P, CHUNKS * C_OUT], F32)
                    nc.scalar.copy(out=yt[:], in_=psy[:])
                    nc.sync.dma_start(
                        out=table_loc[:, H:TBLW].rearrange("(r p) w -> p r w", p=P),
                        in_=yt[:].rearrange("p (r w) -> p r w", r=CHUNKS))

                if phase_lim >= 2:
                    # ===== AG1: all-gather the [N, H+C] table =====
                    nc.gpsimd.collective_compute(
                        "AllGather", mybir.AluOpType.bypass,
                        replica_groups=[list(range(NCORES))],
                        ins=[table_loc.opt()], outs=[table_g.opt()],
                    )

                if phase_lim >= 3:
                    # ===== P2: aggT = T^T @ A^T  (dense, transposed SpMM) =====
                    # out p-tiles: d=0..127 (f1 lo), d=128..255 (f1 hi),
                    # d=256..295 (y -> A@y).  64 k-tiles over all N nodes.
                    aggT = sp.tile([P, HKT * ROWS_PER_CORE], F32,
                                   name=f"aggT_{rep}")
                    aggyT = sp.tile([P, ROWS_PER_CORE], F32,
                                    name=f"aggyT_{rep}")
                    with tc.tile_pool(name=f"p2_{rep}", bufs=1) as p2, \
                         tc.tile_pool(name=f"p2a_{rep}", bufs=2) as p2a, \
                         tc.tile_pool(name=f"p2ps_{rep}", bufs=1, space="PSUM") as p2ps:
                        tbl = p2.tile([P, NKT * TBLW], F32)
                        nc.sync.dma_start(
                            out=tbl[:].rearrange("p (t w) -> p t w", t=NKT),
                            in_=table_g[:].rearrange("(t p) w -> p t w", p=P))
                        ps0 = p2ps.tile([P, ROWS_PER_CORE], F32, space="PSUM")
                        ps1 = p2ps.tile([P, ROWS_PER_CORE], F32, space="PSUM")
                        ps2 = p2ps.tile([P, ROWS_PER_CORE], F32, space="PSUM")
                        pss = [ps0, ps1, ps2]
                        for ab in range(NKT // ABAT):
                            at = p2a.tile([P, ABAT * ROWS_PER_CORE], F32,
                                          tag="at", name=f"at_{ab}")
                            nc.sync.dma_start(
                                out=at[:].rearrange("p (t r) -> p t r", t=ABAT),
                                in_=at_dev[ab * ABAT * P:(ab + 1) * ABAT * P, :]
                                    .rearrange("(t p) r -> p t r", p=P))
                            for j in range(ABAT):
                                kt = ab * ABAT + j
                                for pt in range(3):
                                    dlo = pt * P
                                    dw = P if pt < 2 else C_OUT
                                    for hf in range(2):
                                        nc.tensor.matmul(
                                            out=pss[pt][0:dw, hf * SIMW:
                                                        (hf + 1) * SIMW],
                                            lhsT=tbl[:, kt * TBLW + dlo:
                                                     kt * TBLW + dlo + dw],
                                            rhs=at[:, j * ROWS_PER_CORE
                                                   + hf * SIMW:
                                                   j * ROWS_PER_CORE
                                                   + (hf + 1) * SIMW],
                                            start=(kt == 0), stop=(kt == NKT - 1))
                        nc.scalar.copy(out=aggT[:, 0:ROWS_PER_CORE], in_=ps0[:])
                        nc.scalar.copy(out=aggT[:, ROWS_PER_CORE:], in_=ps1[:])
                        nc.scalar.copy(out=aggyT[:], in_=ps2[:])
                        # A@y back to row-major for the final combine
                        psb = p2ps.tile([P, CHUNKS * C_OUT], F32, space="PSUM")
                        for rt in range(CHUNKS):
                            nc.tensor.transpose(
                                out=psb[:, rt * C_OUT:(rt + 1) * C_OUT],
                                in_=aggyT[0:C_OUT, rt * P:(rt + 1) * P],
                                identity=ident[0:C_OUT, 0:C_OUT])
                        nc.scalar.copy(out=ot1[:], in_=psb[:])

                if phase_lim >= 4:
                    # ===== P3: embT = normalize(balT * [f1T | aggT]) =====
                    with tc.tile_pool(name=f"p3_{rep}", bufs=1) as p3, \
                         tc.tile_pool(name=f"p3ps_{rep}", bufs=1, space="PSUM") as p3ps:
                        e4 = embTloc[:].rearrange("p (k r) -> p k r", k=KT)
                        nc.vector.tensor_tensor(
                            out=e4[:, 0:HKT, :],
                            in0=f1T[:].rearrange("p (k r) -> p k r", k=HKT),
                            in1=balT[:, 0:HKT, None].to_broadcast(
                                [P, HKT, ROWS_PER_CORE]),
                            op=mybir.AluOpType.mult)
                        nc.vector.tensor_tensor(
                            out=e4[:, HKT:KT, :],
                            in0=aggT[:].rearrange("p (k r) -> p k r", k=HKT),
                            in1=balT[:, HKT:KT, None].to_broadcast(
                                [P, HKT, ROWS_PER_CORE]),
                            op=mybir.AluOpType.mult)
                        sq = p3.tile([P, KT * ROWS_PER_CORE], F32)
                        nc.scalar.activation(
                            out=sq[:], in_=embTloc[:],
                            func=mybir.ActivationFunctionType.Square)
                        psn = p3ps.tile([P, ROWS_PER_CORE], F32, space="PSUM")
                        for hf in range(2):
                            for k in range(KT):
                                nc.tensor.matmul(
                                    out=psn[0:1, hf * SIMW:(hf + 1) * SIMW],
                                    lhsT=ones[:, 0:1],
                                    rhs=sq[:, k * ROWS_PER_CORE + hf * SIMW:
                                           k * ROWS_PER_CORE + hf * SIMW + SIMW],
                                    start=(k == 0), stop=(k == KT - 1))
                        nrm = p3.tile([P, ROWS_PER_CORE], F32)
                        nc.scalar.sqrt(out=nrm[0:1, :], in_=psn[0:1, :])
                        nc.vector.tensor_scalar(out=nrm[0:1, :], in0=nrm[0:1, :],
                                                scalar1=1e-8, scalar2=None,
                                                op0=mybir.AluOpType.add)
                        nc.vector.reciprocal(out=nrm[0:1, :], in_=nrm[0:1, :])
                        psb = p3ps.tile([P, ROWS_PER_CORE], F32, space="PSUM")
                        for hf in range(2):
                            nc.tensor.matmul(
                                out=psb[:, hf * SIMW:(hf + 1) * SIMW],
                                lhsT=ones[0:1, :],
                                rhs=nrm[0:1, hf * SIMW:(hf + 1) * SIMW],
                                start=True, stop=True)
                        nb = p3.tile([P, ROWS_PER_CORE], F32)
                        nc.scalar.copy(out=nb[:], in_=psb[:])
                        nc.vector.tensor_tensor(
                            out=e4, in0=e4,
                            in1=nb[:, None, :].to_broadcast(
                                [P, KT, ROWS_PER_CORE]),
                            op=mybir.AluOpType.mult)
                        nc.sync.dma_start(
                            out=embT_loc_d[:].rearrange("(k p) r -> p k r", p=P),
                            in_=e4)

                    # ===== AG2: all-gather transposed embedding =====
                    nc.gpsimd.collective_compute(
                        "AllGather", mybir.AluOpType.bypass,
                        replica_groups=[list(range(NCORES))],
                        ins=[embT_loc_d.opt()], outs=[embT_g.opt()],
                    )

                _sctx.__exit__(None, None, None)

                if phase_lim >= 5:
                    # ===== P4+P5: sim row-block, top-k, combine =====
                    ngroups = CHUNKS // GROUP
                    with tc.tile_pool(name=f"p4rhs_{rep}", bufs=1) as p4rhs, \
                         tc.tile_pool(name=f"p4sim_{rep}", bufs=GROUP) as p4sim, \
                         tc.tile_pool(name=f"p4s_{rep}", bufs=1) as p4s, \
                         tc.tile_pool(name=f"p4ps_{rep}", bufs=1, space="PSUM") as p4ps:
                        m16a = p4s.tile([P, CHUNKS * K_TOP], F32)
                        i16a = p4s.tile([P, CHUNKS * K_TOP], U32)
                        for g in range(ngroups):
                            rts = [g * GROUP + j for j in range(GROUP)]
                            sims = {rt: p4sim.tile([P, N], F32, tag="simbuf",
                                                   name=f"sim_rt{rt}")
                                    for rt in rts}
                            for cg in range(2):
                                rh = p4rhs.tile([P, 4 * KT * ROWS_PER_CORE], F32,
                                                tag="rhs", name=f"rh{g}_{cg}")
                                nc.sync.dma_start(
                                    out=rh[:].rearrange(
                                        "p (b k w) -> p b k w", b=4, k=KT),
                                    in_=embT_g[cg * 8 * H:(cg + 1) * 8 * H, :]
                                        .rearrange("(b k p) w -> p b k w",
                                                   b=4, p=P))
                                rh4 = rh[:].rearrange(
                                    "p (b k w) -> p b k w", b=4, k=KT)
                                for rt in rts:
                                    pss = p4ps.tile([P, 4096], F32,
                                                    space="PSUM", tag="pss",
                                                    name=f"pss{rt}")
                                    for ccq in range(8):
                                        b, off = ccq // 2, (ccq % 2) * SIMW
                                        for k in range(KT):
                                            nc.tensor.matmul(
                                                out=pss[:, ccq * SIMW:
                                                        (ccq + 1) * SIMW],
                                                lhsT=embTloc[
                                                    :, k * ROWS_PER_CORE + rt * P:
                                                    k * ROWS_PER_CORE + (rt + 1) * P],
                                                rhs=rh4[:, b, k, off:off + SIMW],
                                                start=(k == 0), stop=(k == KT - 1))
                                    nc.scalar.copy(
                                        out=sims[rt][:, cg * 4096:(cg + 1) * 4096],
                                        in_=pss[:])
                            for rt in rts:
                                sim = sims[rt]
                                if phase_lim < 6:
                                    nc.vector.max(out=m16a[:, rt * K_TOP:
                                                          rt * K_TOP + 8],
                                                  in_=sim[:])
                                    continue
                                m16 = m16a[:, rt * K_TOP:(rt + 1) * K_TOP]
                                i16 = i16a[:, rt * K_TOP:(rt + 1) * K_TOP]
                                nc.vector.max(out=m16[:, 0:8], in_=sim[:])
                                nc.vector.max_index(out=i16[:, 0:8],
                                                    in_max=m16[:, 0:8],
                                                    in_values=sim[:])
                                nc.vector.match_replace(out=sim[:],
                                                        in_to_replace=m16[:, 0:8],
                                                        in_values=sim[:],
                                                        imm_value=-1e30)
                                nc.vector.max(out=m16[:, 8:16], in_=sim[:])
                                nc.vector.max_index(out=i16[:, 8:16],
                                                    in_max=m16[:, 8:16],
                                                    in_values=sim[:])

                        if phase_lim >= 6:
                            # ===== P5: out = relu(0.5*(Ay + sum relu(v)*y[idx]) + b)
                            v16 = p4s.tile([P, CHUNKS * K_TOP], F32)
                            nc.vector.tensor_scalar(out=v16[:], in0=m16a[:],
                                                    scalar1=0.0, scalar2=None,
                                                    op0=mybir.AluOpType.max)
                            ot = p4s.tile([P, CHUNKS * C_OUT], F32)
                            HG = CHUNKS // 2
                            for hh in range(2):
                                y16 = p4s.tile([P, HG * K_TOP * C_OUT], F32,
                                               tag="y16", name=f"y16_{hh}")
                                for jj in range(HG * K_TOP):
                                    j = hh * HG * K_TOP + jj
                                    nc.gpsimd.indirect_dma_start(
                                        out=y16[:, jj * C_OUT:(jj + 1) * C_OUT],
                                        out_offset=None,
                                        in_=table_g[:, :],
                                        in_offset=bass.IndirectOffsetOnAxis(
                                            ap=i16a[:, j:j + 1], axis=0),
                                        element_offset=H,
                                    )
                                nc.vector.tensor_tensor(
                                    out=y16[:].rearrange("p (a b) -> p a b",
                                                         a=HG * K_TOP),
                                    in0=y16[:].rearrange("p (a b) -> p a b",
                                                         a=HG * K_TOP),
                                    in1=v16[:, hh * HG * K_TOP:
                                            (hh + 1) * HG * K_TOP, None]
                                        .to_broadcast([P, HG * K_TOP, C_OUT]),
                                    op=mybir.AluOpType.mult)
                                for q in range(HG):
                                    rt = hh * HG + q
                                    nc.vector.tensor_reduce(
                                        out=ot[:, rt * C_OUT:(rt + 1) * C_OUT],
                                        in_=y16[:, q * K_TOP * C_OUT:
                                                (q + 1) * K_TOP * C_OUT]
                                            .rearrange("p (a b) -> p b a",
                                                       a=K_TOP),
                                        axis=mybir.AxisListType.X,
                                        op=mybir.AluOpType.add)
                            o3 = ot[:].rearrange("p (r w) -> p r w", r=CHUNKS)
                            nc.vector.tensor_tensor(
                                out=o3, in0=o3,
                                in1=ot1[:].rearrange("p (r w) -> p r w",
                                                     r=CHUNKS),
                                op=mybir.AluOpType.add)
                            nc.vector.tensor_scalar(out=ot[:], in0=ot[:],
                                                    scalar1=0.5, scalar2=None,
                                                    op0=mybir.AluOpType.mult)
                            nc.vector.tensor_tensor(
                                out=o3, in0=o3,
                                in1=bias[:, None, :].to_broadcast(
                                    [P, CHUNKS, C_OUT]),
                                op=mybir.AluOpType.add)
                            nc.vector.tensor_scalar(out=ot[:], in0=ot[:],
                                                    scalar1=0.0, scalar2=None,
                                                    op0=mybir.AluOpType.max)
                            nc.sync.dma_start(
                                out=out_d[:].rearrange("(r p) w -> p r w", p=P),
                                in_=o3)

            if phase_lim < 6:
                with tc.tile_pool(name="dummyout", bufs=1) as dop:
                    zz = dop.tile([P, C_OUT], F32)
                    nc.vector.memset(zz[:], 0.0)
                    nc.sync.dma_start(
                        out=out_d[:].rearrange("(r p) w -> p r w", p=P),
                        in_=zz[:, None, :].to_broadcast([P, CHUNKS, C_OUT]))

    return nc


def prep_inputs(features, adj_rows, adj_cols, adj_vals, tokens, wp_weight,
                global_token, pre_token_w, combine_w, balance_w, gcn_w, gcn_b):
    """Host-side sharding: row-block features; ship dense A^T column-blocks
    (A coalesced with duplicate edges summed); pre-broadcast small params."""
    features = np.ascontiguousarray(np.asarray(features, dtype=np.float32))
    r = np.asarray(adj_rows).astype(np.int64)
    c = np.asarray(adj_cols).astype(np.int64)
    v = np.asarray(adj_vals, dtype=np.float32)

    pre_token = (np.asarray(wp_weight, np.float32) @
                 np.asarray(tokens, np.float32)).reshape(-1)
    cw = np.asarray(combine_w, np.float32).reshape(-1)
    cA = (cw[0] * np.asarray(global_token, np.float32)).reshape(-1)
    cB = pre_token
    cC = (cw[1] * np.asarray(pre_token_w, np.float32)).reshape(-1)
    bal = np.asarray(balance_w, np.float32).reshape(-1)
    bias = np.asarray(gcn_b, np.float32).reshape(-1)

    bcast = lambda x: np.ascontiguousarray(np.tile(x[None, :], (P, 1)))
    gcnw = np.ascontiguousarray(np.asarray(gcn_w, np.float32))
    ident = np.eye(P, dtype=np.float32)
    onesm = np.ones((P, P), dtype=np.float32)
    balT = np.ascontiguousarray(bal.reshape(KT, P).T)   # balT[p, k]

    iota = np.tile(np.arange(P, dtype=np.float32)[None, :], (P, 1))
    iota1k = np.tile(np.arange(ROWS_PER_CORE, dtype=np.float32)[None, :],
                     (P, 1))

    # bucket edges by (dest core, source-column k-tile); coalesce duplicates
    core_of = r // ROWS_PER_CORE
    ct_of = c // P
    key = (core_of * NKT + ct_of) * (N * ROWS_PER_CORE) \
        + (c % P) * ROWS_PER_CORE + (r % ROWS_PER_CORE)
    order = np.argsort(key, kind="stable")
    ks, vs = key[order], v[order]
    uk, inv_idx = np.unique(ks, return_inverse=True)
    uv = np.zeros(len(uk), np.float32)
    np.add.at(uv, inv_idx, vs)
    bucket = uk // (N * ROWS_PER_CORE)
    clocal = (uk % (N * ROWS_PER_CORE)) // ROWS_PER_CORE
    rlocal = uk % ROWS_PER_CORE
    bcounts = np.bincount(bucket, minlength=NCORES * NKT)
    TC = max(1, int(np.ceil(bcounts.max() / P)))
    bec = np.full((NCORES, NKT, TC * P), -1.0, np.float32)
    ber = np.full((NCORES, NKT, TC * P), -1.0, np.float32)
    bev = np.zeros((NCORES, NKT, TC * P), np.float32)
    starts = np.concatenate([[0], np.cumsum(bcounts)])
    for b in range(NCORES * NKT):
        core, ct = b // NKT, b % NKT
        s, e = starts[b], starts[b + 1]
        cnt = e - s
        bec[core, ct, :cnt] = clocal[s:e].astype(np.float32)
        ber[core, ct, :cnt] = rlocal[s:e].astype(np.float32)
        bev[core, ct, :cnt] = uv[s:e]
    # [NKT, TC*P] -> [P, NKT*TC]: edge t*P+p of tile ct at [p, ct*TC+t]
    def shuffle(a):
        return np.ascontiguousarray(
            a.reshape(NCORES, NKT, TC, P).transpose(0, 3, 1, 2)
             .reshape(NCORES, P, NKT * TC))
    bec, ber, bev = shuffle(bec), shuffle(ber), shuffle(bev)

    in_maps = []
    for core in range(NCORES):
        rows = slice(core * ROWS_PER_CORE, (core + 1) * ROWS_PER_CORE)
        in_maps.append({
            "feat": features[rows],
            "bec": bec[core], "ber": ber[core], "bev": bev[core],
            "iota": iota, "iota1k": iota1k,
            "cA": bcast(cA), "cB": bcast(cB), "cC": bcast(cC),
            "balT": balT, "bias": bcast(bias),
            "gcnw": gcnw, "ident": ident, "ones": onesm,
        })
    return in_maps, TC


_BUILD_CACHE = {}


def kernel(features, adj_rows, adj_cols, adj_vals, down_k,
           tokens, wp_weight, global_token, pre_token_w, combine_w,
           balance_w, gcn_w, gcn_b):
    k = int(np.asarray(down_k))
    assert k == K_TOP, f"kernel hardcodes top-k={K_TOP}, got {k}"
    in_maps, T = prep_inputs(features, adj_rows, adj_cols, adj_vals, tokens,
                             wp_weight, global_token, pre_token_w, combine_w,
                             balance_w, gcn_w, gcn_b)
    if T not in _BUILD_CACHE:
        nc_new = build(T)
        _split_waits(nc_new)
        _BUILD_CACHE[T] = nc_new
    nc = _BUILD_CACHE[T]
    res = run_bass_kernel_spmd(nc, in_maps, list(range(NCORES)))
    out = np.concatenate([res.results[i]["out"] for i in range(NCORES)], axis=0)
    return out.astype(np.float32)


# revision 6
# speedup vs baseline: 2.3740x; 1.5769x over previous
"""Trainium2 Bass kernel v4 for nn_DownModel (GNN message passing).

This stack charges ~50us per dynamic instruction (software sequencer),
so the design minimizes dynamic instruction count.  v4: the edge
segment-sum is a dense transposed SpMM  aggT = T^T @ A^T  (A^T shipped
dense from the host, 384 matmuls, no per-edge gathers), which also
yields the aggregate already transposed so the embedding/normalization
runs entirely in transposed layout (norms via ones-vector matmuls).
Row-block sharding over 8 cores; 2 AllGathers (table, embT).
"""

import numpy as np

import concourse.bass as bass
import concourse.mybir as mybir
import concourse.tile as tile
from concourse.bass_utils import run_bass_kernel_spmd

F32 = mybir.dt.float32
I32 = mybir.dt.int32
U32 = mybir.dt.uint32

N = 8192
H = 256
C_OUT = 40
K_TOP = 16
NCORES = 8
P = 128
ROWS_PER_CORE = N // NCORES          # 1024
CHUNKS = ROWS_PER_CORE // P          # 8
KT = (2 * H) // P                    # 4 k-tiles of the 2H embedding dim
HKT = H // P                         # 2 k-tiles of the H dim
TBLW = H + C_OUT                     # 296 table row width
SIMW = 512                           # one PSUM bank of fp32
GROUP = 2                            # row-tiles sharing one rhs stream pass
NKT = N // P                         # 64 k-tiles over all nodes
ABAT = 8                             # A^T k-tiles streamed per DMA


def _split_waits(nc, maxw=1):
    """Walrus accepts one sync-wait per instruction; hoist extras onto NOPs."""
    n_new = 0
    for bb in nc.main_func.blocks:
        new_insts = []
        for ins in bb.instructions:
            si = ins.sync_info
            if si is not None and si.on_wait and len(si.on_wait) > maxw:
                waits = list(si.on_wait)
                excess, keep = waits[:-maxw], waits[-maxw:]
                for i in range(0, len(excess), maxw):
                    nop = mybir.InstNoOp(
                        name=f"waitnop-{ins.name}-{i}",
                        engine=ins.engine,
                        ins=[],
                        outs=[],
                        sync_info=mybir.SyncInfo(
                            on_wait=excess[i:i + maxw], on_update=[]
                        ),
                    )
                    new_insts.append(nop)
                    n_new += 1
                si.on_wait = keep
            new_insts.append(ins)
        bb.instructions[:] = new_insts
    return n_new


def build(TC, repeat=1, phase_lim=6):
    """TC: edge tiles per column k-tile for the one-time dense A^T build."""
    nc = bass.Bass(num_devices=NCORES)

    feat_d = nc.dram_tensor("feat", [ROWS_PER_CORE, H], F32, kind="ExternalInput")
    bec_d = nc.dram_tensor("bec", [P, NKT * TC], F32, kind="ExternalInput")
    ber_d = nc.dram_tensor("ber", [P, NKT * TC], F32, kind="ExternalInput")
    bev_d = nc.dram_tensor("bev", [P, NKT * TC], F32, kind="ExternalInput")
    iota_d = nc.dram_tensor("iota", [P, P], F32, kind="ExternalInput")
    iota1k_d = nc.dram_tensor("iota1k", [P, ROWS_PER_CORE], F32,
                              kind="ExternalInput")
    cA_d = nc.dram_tensor("cA", [P, H], F32, kind="ExternalInput")
    cB_d = nc.dram_tensor("cB", [P, H], F32, kind="ExternalInput")
    cC_d = nc.dram_tensor("cC", [P, H], F32, kind="ExternalInput")
    balT_d = nc.dram_tensor("balT", [P, KT], F32, kind="ExternalInput")
    bias_d = nc.dram_tensor("bias", [P, C_OUT], F32, kind="ExternalInput")
    gcnw_d = nc.dram_tensor("gcnw", [H, C_OUT], F32, kind="ExternalInput")
    ident_d = nc.dram_tensor("ident", [P, P], F32, kind="ExternalInput")
    ones_d = nc.dram_tensor("ones", [P, P], F32, kind="ExternalInput")

    out_d = nc.dram_tensor("out", [ROWS_PER_CORE, C_OUT], F32,
                           kind="ExternalOutput")

    with tile.TileContext(nc) as tc:
        with tc.tile_pool(name="consts", bufs=1) as cp, \
             tc.tile_pool(name="persist", bufs=1) as pp, \
             tc.tile_pool(name="dram", bufs=1, space="DRAM") as dp:

            cA = cp.tile([P, H], F32)
            cB = cp.tile([P, H], F32)
            cC = cp.tile([P, H], F32)
            balT = cp.tile([P, KT], F32)
            bias = cp.tile([P, C_OUT], F32)
            gcnw = cp.tile([P, HKT * C_OUT], F32)
            ident = cp.tile([P, P], F32)
            ones = cp.tile([P, P], F32)
            nc.sync.dma_start(out=cA[:], in_=cA_d[:])
            nc.sync.dma_start(out=cB[:], in_=cB_d[:])
            nc.sync.dma_start(out=cC[:], in_=cC_d[:])
            nc.sync.dma_start(out=balT[:], in_=balT_d[:])
            nc.sync.dma_start(out=bias[:], in_=bias_d[:])
            nc.sync.dma_start(
                out=gcnw[:].rearrange("p (k w) -> p k w", k=HKT),
                in_=gcnw_d[:].rearrange("(k p) w -> p k w", p=P))
            nc.sync.dma_start(out=ident[:], in_=ident_d[:])
            nc.sync.dma_start(out=ones[:], in_=ones_d[:])

            embTloc = pp.tile([P, KT * ROWS_PER_CORE], F32)
            ot1 = pp.tile([P, CHUNKS * C_OUT], F32)   # A@y, row-major

            # ===== one-time: dense A^T built on device from one-hot edges
            # (outside the repeat loop; constant across reps) =====
            at_dev = dp.tile([N, ROWS_PER_CORE], F32, name="at_dev")
            with tc.tile_pool(name="bld", bufs=1) as bp, \
                 tc.tile_pool(name="bldps", bufs=2, space="PSUM") as bps:
                bec = bp.tile([P, NKT * TC], F32)
                ber = bp.tile([P, NKT * TC], F32)
                bev = bp.tile([P, NKT * TC], F32)
                iota = bp.tile([P, P], F32)
                iota1k = bp.tile([P, ROWS_PER_CORE], F32)
                nc.sync.dma_start(out=bec[:], in_=bec_d[:])
                nc.sync.dma_start(out=ber[:], in_=ber_d[:])
                nc.sync.dma_start(out=bev[:], in_=bev_d[:])
                nc.sync.dma_start(out=iota[:], in_=iota_d[:])
                nc.sync.dma_start(out=iota1k[:], in_=iota1k_d[:])
                for ct in range(NKT):
                    Sc = bp.tile([P, TC * P], F32, tag="sc", name=f"sc{ct}")
                    nc.vector.tensor_tensor(
                        out=Sc[:].rearrange("p (t c) -> p t c", t=TC),
                        in0=bec[:, ct * TC:(ct + 1) * TC, None]
                            .to_broadcast([P, TC, P]),
                        in1=iota[:, None, :].to_broadcast([P, TC, P]),
                        op=mybir.AluOpType.is_equal)
                    Rv = bp.tile([P, TC * ROWS_PER_CORE], F32, tag="rv",
                                 name=f"rv{ct}")
                    R3 = Rv[:].rearrange("p (t r) -> p t r", t=TC)
                    nc.vector.tensor_tensor(
                        out=R3,
                        in0=ber[:, ct * TC:(ct + 1) * TC, None]
                            .to_broadcast([P, TC, ROWS_PER_CORE]),
                        in1=iota1k[:, None, :].to_broadcast(
                            [P, TC, ROWS_PER_CORE]),
                        op=mybir.AluOpType.is_equal)
                    nc.vector.tensor_tensor(
                        out=R3, in0=R3,
                        in1=bev[:, ct * TC:(ct + 1) * TC, None]
                            .to_broadcast([P, TC, ROWS_PER_CORE]),
                        op=mybir.AluOpType.mult)
                    psA = bps.tile([P, ROWS_PER_CORE], F32, space="PSUM",
                                   tag="psA", name=f"psA{ct}")
                    for t in range(TC):
                        for hf in range(2):
                            nc.tensor.matmul(
                                out=psA[:, hf * SIMW:(hf + 1) * SIMW],
                                lhsT=Sc[:, t * P:(t + 1) * P],
                                rhs=Rv[:, t * ROWS_PER_CORE + hf * SIMW:
                                       t * ROWS_PER_CORE + hf * SIMW + SIMW],
                                start=(t == 0), stop=(t == TC - 1))
                    ab = bp.tile([P, ROWS_PER_CORE], F32, tag="ab",
                                 name=f"ab{ct}")
                    nc.scalar.copy(out=ab[:], in_=psA[:])
                    nc.sync.dma_start(out=at_dev[ct * P:(ct + 1) * P, :],
                                      in_=ab[:])

            for rep in range(repeat):
                table_loc = dp.tile([ROWS_PER_CORE, TBLW], F32,
                                    name=f"table_loc_{rep}")
                table_g = dp.tile([N, TBLW], F32, addr_space="Shared",
                                  name=f"table_g_{rep}")
                embT_loc_d = dp.tile([2 * H, ROWS_PER_CORE], F32,
                                     name=f"embT_loc_d_{rep}")
                embT_g = dp.tile([NCORES * 2 * H, ROWS_PER_CORE], F32,
                                 addr_space="Shared", name=f"embT_g_{rep}")

                _sctx = tc.tile_pool(name=f"sp_{rep}", bufs=1)
                sp = _sctx.__enter__()
                f1T = sp.tile([P, HKT * ROWS_PER_CORE], F32,
                              name=f"f1T_{rep}")     # [p, k(2), r(1024)]

                # ===== P1: f1 (row-major) + f1T + y for the local block =====
                with tc.tile_pool(name=f"p1_{rep}", bufs=1) as p1, \
                     tc.tile_pool(name=f"p1ps_{rep}", bufs=1, space="PSUM") as p1ps:
                    W1 = CHUNKS * H
                    ft = p1.tile([P, W1], F32)
                    nc.sync.dma_start(
                        out=ft[:].rearrange("p (r w) -> p r w", r=CHUNKS),
                        in_=feat_d[:].rearrange("(r p) w -> p r w", p=P))
                    mB = lambda c: c[:, None, :].to_broadcast([P, CHUNKS, H])
                    v3 = lambda t: t[:].rearrange("p (r w) -> p r w", r=CHUNKS)
                    m = p1.tile([P, W1], F32)
                    nc.vector.tensor_tensor(out=v3(m), in0=v3(ft), in1=mB(cB),
                                            op=mybir.AluOpType.mult)
                    nc.vector.tensor_scalar(out=m[:], in0=m[:], scalar1=0.0,
                                            scalar2=None, op0=mybir.AluOpType.max)
                    nc.vector.tensor_tensor(out=v3(m), in0=v3(m), in1=mB(cA),
                                            op=mybir.AluOpType.mult)
                    v = p1.tile([P, W1], F32)
                    nc.vector.tensor_tensor(out=v3(v), in0=v3(ft), in1=mB(cC),
                                            op=mybir.AluOpType.mult)
                    nc.vector.tensor_tensor(out=m[:], in0=m[:], in1=v[:],
                                            op=mybir.AluOpType.add)
                    # elu(z) = (relu(z) - 1) + exp(min(z, 0))
                    nc.vector.tensor_scalar(out=v[:], in0=m[:], scalar1=0.0,
                                            scalar2=-1.0, op0=mybir.AluOpType.max,
                                            op1=mybir.AluOpType.add)
                    nc.vector.tensor_scalar(out=m[:], in0=m[:], scalar1=0.0,
                                            scalar2=None, op0=mybir.AluOpType.min)
                    nc.scalar.activation(out=m[:], in_=m[:],
                                         func=mybir.ActivationFunctionType.Exp)
                    f1_all = p1.tile([P, W1], F32)
                    nc.vector.tensor_tensor(out=f1_all[:], in0=v[:], in1=m[:],
                                            op=mybir.AluOpType.add)
                    nc.sync.dma_start(
                        out=table_loc[:, 0:H].rearrange("(r p) w -> p r w", p=P),
                        in_=v3(f1_all))
                    # 16 transposes -> f1T in [p, k, rt*128+c] layout
                    psT = p1ps.tile([P, 2048], F32, space="PSUM")
                    for rt in range(CHUNKS):
                        for k in range(HKT):
                            nc.tensor.transpose(
                                out=psT[:, (rt * HKT + k) * P:
                                        (rt * HKT + k + 1) * P],
                                in_=f1_all[:, rt * H + k * P:rt * H + (k + 1) * P],
                                identity=ident[:])
                    nc.scalar.copy(
                        out=f1T[:].rearrange("p (k r c) -> p r k c",
                                             k=HKT, r=CHUNKS),
                        in_=psT[:].rearrange("p (r k c) -> p r k c",
                                             r=CHUNKS, k=HKT))
                    # y = f1 @ gcn_w
                    psy = p1ps.tile([P, CHUNKS * C_OUT], F32, space="PSUM")
                    for rt in range(CHUNKS):
                        for k in range(HKT):
                            nc.tensor.matmul(
                                out=psy[:, rt * C_OUT:(rt + 1) * C_OUT],
                                lhsT=f1T[:, k * ROWS_PER_CORE + rt * P:
                                         k * ROWS_PER_CORE + (rt + 1) * P],
                                rhs=gcnw[:, k * C_OUT:(k + 1) * C_OUT],
                                start=(k == 0), stop=(k == HKT - 1))
                    yt = p1.tile([P, CHUNKS * C_OUT], F32)
                    nc.scalar.copy(out=yt[:], in_=psy[:])
                    nc.sync.dma_start(
                        out=table_loc[:, H:TBLW].rearrange("(r p) w -> p r w", p=P),
                        in_=yt[:].rearrange("p (r w) -> p r w", r=CHUNKS))

                if phase_lim >= 2:
                    # ===== AG1: all-gather the [N, H+C] table =====
                    nc.gpsimd.collective_compute(
                        "AllGather", mybir.AluOpType.bypass,
                        replica_groups=[list(range(NCORES))],
                        ins=[table_loc.opt()], outs=[table_g.opt()],
                    )

                if phase_lim >= 3:
                    # ===== P2: aggT = T^T @ A^T  (dense, transposed SpMM) =====
                    # out p-tiles: d=0..127 (f1 lo), d=128..255 (f1 hi),
                    # d=256..295 (y -> A@y).  64 k-tiles over all N nodes.
                    aggT = sp.tile([P, HKT * ROWS_PER_CORE], F32,
                                   name=f"aggT_{rep}")
                    with tc.tile_pool(name=f"p2_{rep}", bufs=1) as p2, \
                         tc.tile_pool(name=f"p2a_{rep}", bufs=2) as p2a, \
                         tc.tile_pool(name=f"p2ps_{rep}", bufs=1, space="PSUM") as p2ps:
                        tbl = p2.tile([P, NKT * TBLW], F32)
                        nc.sync.dma_start(
                            out=tbl[:].rearrange("p (t w) -> p t w", t=NKT),
                            in_=table_g[:].rearrange("(t p) w -> p t w", p=P))
                        ps0 = p2ps.tile([P, ROWS_PER_CORE], F32, space="PSUM")
                        ps1 = p2ps.tile([P, ROWS_PER_CORE], F32, space="PSUM")
                        pss = [ps0, ps1]
                        for ab in range(NKT // ABAT):
                            at = p2a.tile([P, ABAT * ROWS_PER_CORE], F32,
                                          tag="at", name=f"at_{ab}")
                            nc.sync.dma_start(
                                out=at[:].rearrange("p (t r) -> p t r", t=ABAT),
                                in_=at_dev[ab * ABAT * P:(ab + 1) * ABAT * P, :]
                                    .rearrange("(t p) r -> p t r", p=P))
                            for j in range(ABAT):
                                kt = ab * ABAT + j
                                for pt in range(2):
                                    dlo = pt * P
                                    dw = P
                                    for hf in range(2):
                                        nc.tensor.matmul(
                                            out=pss[pt][0:dw, hf * SIMW:
                                                        (hf + 1) * SIMW],
                                            lhsT=tbl[:, kt * TBLW + dlo:
                                                     kt * TBLW + dlo + dw],
                                            rhs=at[:, j * ROWS_PER_CORE
                                                   + hf * SIMW:
                                                   j * ROWS_PER_CORE
                                                   + (hf + 1) * SIMW],
                                            start=(kt == 0), stop=(kt == NKT - 1))
                        nc.scalar.copy(out=aggT[:, 0:ROWS_PER_CORE], in_=ps0[:])
                        nc.scalar.copy(out=aggT[:, ROWS_PER_CORE:], in_=ps1[:])
                        # out1 = A@y = (A@f1)@W, with aggT as ready-made lhsT
                        psb = p2ps.tile([P, CHUNKS * C_OUT], F32, space="PSUM")
                        for rt in range(CHUNKS):
                            for k in range(HKT):
                                nc.tensor.matmul(
                                    out=psb[:, rt * C_OUT:(rt + 1) * C_OUT],
                                    lhsT=aggT[:, k * ROWS_PER_CORE + rt * P:
                                              k * ROWS_PER_CORE + (rt + 1) * P],
                                    rhs=gcnw[:, k * C_OUT:(k + 1) * C_OUT],
                                    start=(k == 0), stop=(k == HKT - 1))
                        nc.scalar.copy(out=ot1[:], in_=psb[:])

                if phase_lim >= 4:
                    # ===== P3: embT = normalize(balT * [f1T | aggT]) =====
                    with tc.tile_pool(name=f"p3_{rep}", bufs=1) as p3, \
                         tc.tile_pool(name=f"p3ps_{rep}", bufs=1, space="PSUM") as p3ps:
                        e4 = embTloc[:].rearrange("p (k r) -> p k r", k=KT)
                        nc.vector.tensor_tensor(
                            out=e4[:, 0:HKT, :],
                            in0=f1T[:].rearrange("p (k r) -> p k r", k=HKT),
                            in1=balT[:, 0:HKT, None].to_broadcast(
                                [P, HKT, ROWS_PER_CORE]),
                            op=mybir.AluOpType.mult)
                        nc.vector.tensor_tensor(
                            out=e4[:, HKT:KT, :],
                            in0=aggT[:].rearrange("p (k r) -> p k r", k=HKT),
                            in1=balT[:, HKT:KT, None].to_broadcast(
                                [P, HKT, ROWS_PER_CORE]),
                            op=mybir.AluOpType.mult)
                        sq = p3.tile([P, KT * ROWS_PER_CORE], F32)
                        nc.scalar.activation(
                            out=sq[:], in_=embTloc[:],
                            func=mybir.ActivationFunctionType.Square)
                        psn = p3ps.tile([P, ROWS_PER_CORE], F32, space="PSUM")
                        for hf in range(2):
                            for k in range(KT):
                                nc.tensor.matmul(
                                    out=psn[0:1, hf * SIMW:(hf + 1) * SIMW],
                                    lhsT=ones[:, 0:1],
                                    rhs=sq[:, k * ROWS_PER_CORE + hf * SIMW:
                                           k * ROWS_PER_CORE + hf * SIMW + SIMW],
                                    start=(k == 0), stop=(k == KT - 1))
                        nrm = p3.tile([P, ROWS_PER_CORE], F32)
                        nc.scalar.sqrt(out=nrm[0:1, :], in_=psn[0:1, :])
                        nc.vector.tensor_scalar(out=nrm[0:1, :], in0=nrm[0:1, :],
                                                scalar1=1e-8, scalar2=None,
                                                op0=mybir.AluOpType.add)
                        nc.vector.reciprocal(out=nrm[0:1, :], in_=nrm[0:1, :])
                        psb = p3ps.tile([P, ROWS_PER_CORE], F32, space="PSUM")
                        for hf in range(2):
                            nc.tensor.matmul(
                                out=psb[:, hf * SIMW:(hf + 1) * SIMW],
                                lhsT=ones[0:1, :],
                                rhs=nrm[0:1, hf * SIMW:(hf + 1) * SIMW],
                                start=True, stop=True)
                        nb = p3.tile([P, ROWS_PER_CORE], F32)
                        nc.scalar.copy(out=nb[:], in_=psb[:])
                        nc.vector.tensor_tensor(
                            out=e4, in0=e4,
                            in1=nb[:, None, :].to_broadcast(
                                [P, KT, ROWS_PER_CORE]),
                            op=mybir.AluOpType.mult)
                        nc.sync.dma_start(
                            out=embT_loc_d[:].rearrange("(k p) r -> p k r", p=P),
                            in_=e4)

                    # ===== AG2: all-gather transposed embedding =====
                    nc.gpsimd.collective_compute(
                        "AllGather", mybir.AluOpType.bypass,
                        replica_groups=[list(range(NCORES))],
                        ins=[embT_loc_d.opt()], outs=[embT_g.opt()],
                    )

                _sctx.__exit__(None, None, None)

                if phase_lim >= 5:
                    # ===== P4+P5: sim row-block, top-k, combine =====
                    ngroups = CHUNKS // GROUP
                    with tc.tile_pool(name=f"p4rhs_{rep}", bufs=1) as p4rhs, \
                         tc.tile_pool(name=f"p4sim_{rep}", bufs=GROUP) as p4sim, \
                         tc.tile_pool(name=f"p4s_{rep}", bufs=1) as p4s, \
                         tc.tile_pool(name=f"p4ps_{rep}", bufs=1, space="PSUM") as p4ps:
                        m16a = p4s.tile([P, CHUNKS * K_TOP], F32)
                        i16a = p4s.tile([P, CHUNKS * K_TOP], U32)
                        for g in range(ngroups):
                            rts = [g * GROUP + j for j in range(GROUP)]
                            sims = {rt: p4sim.tile([P, N], F32, tag="simbuf",
                                                   name=f"sim_rt{rt}")
                                    for rt in rts}
                            for cg in range(2):
                                rh = p4rhs.tile([P, 4 * KT * ROWS_PER_CORE], F32,
                                                tag="rhs", name=f"rh{g}_{cg}")
                                nc.sync.dma_start(
                                    out=rh[:].rearrange(
                                        "p (b k w) -> p b k w", b=4, k=KT),
                                    in_=embT_g[cg * 8 * H:(cg + 1) * 8 * H, :]
                                        .rearrange("(b k p) w -> p b k w",
                                                   b=4, p=P))
                                rh4 = rh[:].rearrange(
                                    "p (b k w) -> p b k w", b=4, k=KT)
                                for rt in rts:
                                    pss = p4ps.tile([P, 4096], F32,
                                                    space="PSUM", tag="pss",
                                                    name=f"pss{rt}")
                                    for ccq in range(8):
                                        b, off = ccq // 2, (ccq % 2) * SIMW
                                        for k in range(KT):
                                            nc.tensor.matmul(
                                                out=pss[:, ccq * SIMW:
                                                        (ccq + 1) * SIMW],
                                                lhsT=embTloc[
                                                    :, k * ROWS_PER_CORE + rt * P:
                                                    k * ROWS_PER_CORE + (rt + 1) * P],
                                                rhs=rh4[:, b, k, off:off + SIMW],
                                                start=(k == 0), stop=(k == KT - 1))
                                    nc.scalar.copy(
                                        out=sims[rt][:, cg * 4096:(cg + 1) * 4096],
                                        in_=pss[:])
                            for rt in rts:
                                sim = sims[rt]
                                if phase_lim < 6:
                                    nc.vector.max(out=m16a[:, rt * K_TOP:
                                                          rt * K_TOP + 8],
                                                  in_=sim[:])
                                    continue
                                m16 = m16a[:, rt * K_TOP:(rt + 1) * K_TOP]
                                i16 = i16a[:, rt * K_TOP:(rt + 1) * K_TOP]
                                nc.vector.max(out=m16[:, 0:8], in_=sim[:])
                                nc.vector.max_index(out=i16[:, 0:8],
                                                    in_max=m16[:, 0:8],
                                                    in_values=sim[:])
                                nc.vector.match_replace(out=sim[:],
                                                        in_to_replace=m16[:, 0:8],
                                                        in_values=sim[:],
                                                        imm_value=-1e30)
                                nc.vector.max(out=m16[:, 8:16], in_=sim[:])
                                nc.vector.max_index(out=i16[:, 8:16],
                                                    in_max=m16[:, 8:16],
                                                    in_values=sim[:])

                        if phase_lim >= 6:
                            # ===== P5: out = relu(0.5*(Ay + sum relu(v)*y[idx]) + b)
                            v16 = p4s.tile([P, CHUNKS * K_TOP], F32)
                            nc.vector.tensor_scalar(out=v16[:], in0=m16a[:],
                                                    scalar1=0.0, scalar2=None,
                                                    op0=mybir.AluOpType.max)
                            ot = p4s.tile([P, CHUNKS * C_OUT], F32)
                            HG = CHUNKS // 2
                            for hh in range(2):
                                y16 = p4s.tile([P, HG * K_TOP * C_OUT], F32,
                                               tag="y16", name=f"y16_{hh}")
                                for jj in range(HG * K_TOP):
                                    j = hh * HG * K_TOP + jj
                                    nc.gpsimd.indirect_dma_start(
                                        out=y16[:, jj * C_OUT:(jj + 1) * C_OUT],
                                        out_offset=None,
                                        in_=table_g[:, :],
                                        in_offset=bass.IndirectOffsetOnAxis(
                                            ap=i16a[:, j:j + 1], axis=0),
                                        element_offset=H,
                                    )
                                nc.vector.tensor_tensor(
                                    out=y16[:].rearrange("p (a b) -> p a b",
                                                         a=HG * K_TOP),
                                    in0=y16[:].rearrange("p (a b) -> p a b",
                                                         a=HG * K_TOP),
                                    in1=v16[:, hh * HG * K_TOP:
                                            (hh + 1) * HG * K_TOP, None]
                                        .to_broadcast([P, HG * K_TOP, C_OUT]),
                                    op=mybir.AluOpType.mult)
                                for q in range(HG):
                                    rt = hh * HG + q
                                    nc.vector.tensor_reduce(
                                        out=ot[:, rt * C_OUT:(rt + 1) * C_OUT],
                                        in_=y16[:, q * K_TOP * C_OUT:
                                                (q + 1) * K_TOP * C_OUT]
                                            .rearrange("p (a b) -> p b a",
                                                       a=K_TOP),
                                        axis=mybir.AxisListType.X,
                                        op=mybir.AluOpType.add)
                            o3 = ot[:].rearrange("p (r w) -> p r w", r=CHUNKS)
                            nc.vector.tensor_tensor(
                                out=o3, in0=o3,
                                in1=ot1[:].rearrange("p (r w) -> p r w",
                                                     r=CHUNKS),
                                op=mybir.AluOpType.add)
                            nc.vector.tensor_scalar(out=ot[:], in0=ot[:],
                                                    scalar1=0.5, scalar2=None,
                                                    op0=mybir.AluOpType.mult)
                            nc.vector.tensor_tensor(
                                out=o3, in0=o3,
                                in1=bias[:, None, :].to_broadcast(
                                    [P, CHUNKS, C_OUT]),
                                op=mybir.AluOpType.add)
                            nc.vector.tensor_scalar(out=ot[:], in0=ot[:],
                                                    scalar1=0.0, scalar2=None,
                                                    op0=mybir.AluOpType.max)
                            nc.sync.dma_start(
                                out=out_d[:].rearrange("(r p) w -> p r w", p=P),
                                in_=o3)

            if phase_lim < 6:
                with tc.tile_pool(name="dummyout", bufs=1) as dop:
                    zz = dop.tile([P, C_OUT], F32)
                    nc.vector.memset(zz[:], 0.0)
                    nc.sync.dma_start(
                        out=out_d[:].rearrange("(r p) w -> p r w", p=P),
                        in_=zz[:, None, :].to_broadcast([P, CHUNKS, C_OUT]))

    return nc


def prep_inputs(features, adj_rows, adj_cols, adj_vals, tokens, wp_weight,
                global_token, pre_token_w, combine_w, balance_w, gcn_w, gcn_b):
    """Host-side sharding: row-block features; ship dense A^T column-blocks
    (A coalesced with duplicate edges summed); pre-broadcast small params."""
    features = np.ascontiguousarray(np.asarray(features, dtype=np.float32))
    r = np.asarray(adj_rows).astype(np.int64)
    c = np.asarray(adj_cols).astype(np.int64)
    v = np.asarray(adj_vals, dtype=np.float32)

    pre_token = (np.asarray(wp_weight, np.float32) @
                 np.asarray(tokens, np.float32)).reshape(-1)
    cw = np.asarray(combine_w, np.float32).reshape(-1)
    cA = (cw[0] * np.asarray(global_token, np.float32)).reshape(-1)
    cB = pre_token
    cC = (cw[1] * np.asarray(pre_token_w, np.float32)).reshape(-1)
    bal = np.asarray(balance_w, np.float32).reshape(-1)
    bias = np.asarray(gcn_b, np.float32).reshape(-1)

    bcast = lambda x: np.ascontiguousarray(np.tile(x[None, :], (P, 1)))
    gcnw = np.ascontiguousarray(np.asarray(gcn_w, np.float32))
    ident = np.eye(P, dtype=np.float32)
    onesm = np.ones((P, P), dtype=np.float32)
    balT = np.ascontiguousarray(bal.reshape(KT, P).T)   # balT[p, k]

    iota = np.tile(np.arange(P, dtype=np.float32)[None, :], (P, 1))
    iota1k = np.tile(np.arange(ROWS_PER_CORE, dtype=np.float32)[None, :],
                     (P, 1))

    # bucket edges by (dest core, source-column k-tile); coalesce duplicates
    core_of = r // ROWS_PER_CORE
    ct_of = c // P
    key = (core_of * NKT + ct_of) * (N * ROWS_PER_CORE) \
        + (c % P) * ROWS_PER_CORE + (r % ROWS_PER_CORE)
    order = np.argsort(key, kind="stable")
    ks, vs = key[order], v[order]
    uk, inv_idx = np.unique(ks, return_inverse=True)
    uv = np.zeros(len(uk), np.float32)
    np.add.at(uv, inv_idx, vs)
    bucket = uk // (N * ROWS_PER_CORE)
    clocal = (uk % (N * ROWS_PER_CORE)) // ROWS_PER_CORE
    rlocal = uk % ROWS_PER_CORE
    bcounts = np.bincount(bucket, minlength=NCORES * NKT)
    TC = max(1, int(np.ceil(bcounts.max() / P)))
    bec = np.full((NCORES, NKT, TC * P), -1.0, np.float32)
    ber = np.full((NCORES, NKT, TC * P), -1.0, np.float32)
    bev = np.zeros((NCORES, NKT, TC * P), np.float32)
    starts = np.concatenate([[0], np.cumsum(bcounts)])
    for b in range(NCORES * NKT):
        core, ct = b // NKT, b % NKT
        s, e = starts[b], starts[b + 1]
        cnt = e - s
        bec[core, ct, :cnt] = clocal[s:e].astype(np.float32)
        ber[core, ct, :cnt] = rlocal[s:e].astype(np.float32)
        bev[core, ct, :cnt] = uv[s:e]
    # [NKT, TC*P] -> [P, NKT*TC]: edge t*P+p of tile ct at [p, ct*TC+t]
    def shuffle(a):
        return np.ascontiguousarray(
            a.reshape(NCORES, NKT, TC, P).transpose(0, 3, 1, 2)
             .reshape(NCORES, P, NKT * TC))
    bec, ber, bev = shuffle(bec), shuffle(ber), shuffle(bev)

    in_maps = []
    for core in range(NCORES):
        rows = slice(core * ROWS_PER_CORE, (core + 1) * ROWS_PER_CORE)
        in_maps.append({
            "feat": features[rows],
            "bec": bec[core], "ber": ber[core], "bev": bev[core],
            "iota": iota, "iota1k": iota1k,
            "cA": bcast(cA), "cB": bcast(cB), "cC": bcast(cC),
            "balT": balT, "bias": bcast(bias),
            "gcnw": gcnw, "ident": ident, "ones": onesm,
        })
    return in_maps, TC


_BUILD_CACHE = {}


def kernel(features, adj_rows, adj_cols, adj_vals, down_k,
           tokens, wp_weight, global_token, pre_token_w, combine_w,
           balance_w, gcn_w, gcn_b):
    k = int(np.asarray(down_k))
    assert k == K_TOP, f"kernel hardcodes top-k={K_TOP}, got {k}"
    in_maps, T = prep_inputs(features, adj_rows, adj_cols, adj_vals, tokens,
                             wp_weight, global_token, pre_token_w, combine_w,
                             balance_w, gcn_w, gcn_b)
    if T not in _BUILD_CACHE:
        nc_new = build(T)
        _split_waits(nc_new)
        _BUILD_CACHE[T] = nc_new
    nc = _BUILD_CACHE[T]
    res = run_bass_kernel_spmd(nc, in_maps, list(range(NCORES)))
    out = np.concatenate([res.results[i]["out"] for i in range(NCORES)], axis=0)
    return out.astype(np.float32)
